# revision 8
# baseline (speedup 1.0000x reference)
"""Trainium2 Bass kernel for nn_AttentionFusion (cross-attention, B=4, LQ=1024,
LKV=4096, D=512, H=4 heads of 128).

Sharding: 8 cores = (batch b in 0..3) x (head-pair hp in 0..1). Core c = 2*b+hp
computes attention for heads {2hp, 2hp+1} of batch b plus its partial
out-projection (tensor-parallel split of Wo). Host sums the two partials per
batch (the TP un-shard); everything else runs on device in bf16 with fp32
accumulation.

v4: activations pre-transposed + pre-cast to bf16 on host (no on-chip
transposes, contiguous loads, halved DMA). bk dropped (a per-q additive score
shift cancels in softmax). All projection / out-projection work is emitted as
small "units" interleaved into the attention tile stream so the PE never
idles on the exp engine (exp ~1.0us/tile vs 864ns of attention MMs). Non-head
loads ride the ACT queue as staggered triggers so the first x/enc chunks get
full DMA bandwidth. Denominator tree pre-collapses at kt30, transposes run in
bf16, and the tail fuses (psum*recip)+carry in one DVE op per q-tile.

Per-core dataflow:
  xT [e,q], eT [e,kv]   <- direct chunked DMA (already bf16+transposed)
  qT [d,q]              <- weight-stationary projection; bq fused on ACT
  kT [d,kv]             <- weight-stationary projection (no bias; DVE copy)
  v  [kv,d]             <- encoder-stationary projection (bv folded into cvec)
  scoresT [kv,q] (PSUM) = kT-tile.T @ qT ; P = exp(scale*scoresT) on ACT (bf16)
  ctx~T [d,q]  (PSUM)  += v-tile.T @ P  over kv tiles (unnormalized, 1-deep
                          software pipeline: MM2(t) emitted after MM1(t+1))
  denom: bf16 pairwise tree of P tiles on DVE -> PE-transpose (bf16) ->
         free-dim reduce -> reciprocal (per-partition [q,1] layout)
  out[q,e] = (ctx~T.T @ Wo_h.T) * recip_h[q] (+ cvec)  summed over h, f32
"""

import numpy as np

B, LQ, LKV, D, H, HD = 4, 1024, 4096, 512, 4, 128
NCORES = 8
SCALE = 1.0 / float(np.sqrt(HD))

_compiled = {}


def _build():
    import concourse.bacc as bacc
    import concourse.mybir as mybir
    from concourse import tile
    from concourse.masks import make_identity

    bf16, f32 = mybir.dt.bfloat16, mybir.dt.float32
    EXP = mybir.ActivationFunctionType.Exp
    IDN = mybir.ActivationFunctionType.Identity
    MUL = mybir.AluOpType.mult
    ADD = mybir.AluOpType.add

    nc = bacc.Bacc(
        "TRN2",
        target_bir_lowering=False,
        debug=False,
        enable_asserts=True,
        num_devices=NCORES,
    )

    xt = nc.dram_tensor("xt", [D, LQ], bf16, kind="ExternalInput")
    et = nc.dram_tensor("et", [D, LKV], bf16, kind="ExternalInput")
    wqt = nc.dram_tensor("wqt", [128, 1024], bf16, kind="ExternalInput")
    wkt = nc.dram_tensor("wkt", [128, 1024], bf16, kind="ExternalInput")
    wvt = nc.dram_tensor("wvt", [128, 1024], bf16, kind="ExternalInput")
    wot = nc.dram_tensor("wot", [128, 1024], bf16, kind="ExternalInput")
    bq2 = nc.dram_tensor("bq2", [128, 2], f32, kind="ExternalInput")
    cvec = nc.dram_tensor("cvec", [D], f32, kind="ExternalInput")
    outp = nc.dram_tensor("outp", [LQ, D], f32, kind="ExternalOutput")

    with tile.TileContext(nc) as tc:
        with (
            tc.tile_pool(name="const", bufs=1) as const,
            tc.tile_pool(name="big", bufs=1) as big,
            tc.tile_pool(name="expp", bufs=6) as expp,
            tc.tile_pool(name="tree", bufs=7) as treep,
            tc.tile_pool(name="smal", bufs=4) as smal,
            tc.tile_pool(name="nrm0p", bufs=8) as nrm0p,
            tc.tile_pool(name="osb", bufs=4) as osb,
            tc.tile_pool(name="ps", bufs=3, space="PSUM") as psp,
            tc.tile_pool(name="ps_c", bufs=1, space="PSUM") as ps_c,
        ):
            wq_sb = const.tile([128, 4, 256], bf16)
            wk_sb = const.tile([128, 4, 256], bf16)
            wv_sb = const.tile([128, 4, 256], bf16)
            wo_sb = const.tile([128, 2, D], bf16)
            bqsb = const.tile([128, 2], f32)

            xT = big.tile([128, 4, LQ], bf16)
            eT = [
                big.tile([128, 4, 1024], bf16, tag=f"eT{g}", name=f"eT{g}")
                for g in range(4)
            ]

            def ld_x(c):
                nc.sync.dma_start(
                    xT[:, :, 512 * c : 512 * c + 512],
                    xt.ap()[:, 512 * c : 512 * c + 512].rearrange(
                        "(k p) q -> p k q", k=4
                    ),
                )

            def ld_e(g, c, eng=None):
                lo = 1024 * g + 512 * c
                (eng or nc.sync).dma_start(
                    eT[g][:, :, 512 * c : 512 * c + 512],
                    et.ap()[:, lo : lo + 512].rearrange("(k p) kv -> p k kv", k=4),
                )

            # sync queue: only what the head critically needs (plus e1c0)
            nc.sync.dma_start(wq_sb[:], wqt.ap().rearrange("p (k d) -> p k d", k=4))
            ld_x(0)
            nc.sync.dma_start(wk_sb[:], wkt.ap().rearrange("p (k d) -> p k d", k=4))
            ld_e(0, 0)
            nc.sync.dma_start(bqsb[:], bq2[:])
            ld_x(1)
            ld_e(0, 1)
            nc.sync.dma_start(wv_sb[:], wvt.ap().rearrange("p (k d) -> p k d", k=4))
            ld_e(1, 0)

            # --- constants ---
            ones = const.tile([128, 1], f32)
            nc.vector.memset(ones[:], 1.0)
            identb = const.tile([128, 128], bf16)
            make_identity(nc, identb[:])
            # warm the ACT exp table set early (~2.7us table load)
            warm = const.tile([128, 1], f32)
            nc.scalar.activation(warm[:], ones[:], EXP)
            # remaining loads ride the ACT queue: each trigger fires when the
            # exp stream reaches it, keeping early DMA bandwidth for x/e0
            ld_e(1, 1, eng=nc.scalar)

            qT = [
                big.tile([128, LQ], bf16, tag=f"qT{h}", name=f"qT{h}")
                for h in range(2)
            ]
            kT = [
                [
                    big.tile([128, 1024], bf16, tag=f"kT{h}_{g}", name=f"kT{h}_{g}")
                    for g in range(4)
                ]
                for h in range(2)
            ]
            v_g = [
                big.tile([128, 8, 256], bf16, tag=f"v{g}", name=f"v{g}")
                for g in range(4)
            ]
            ctxT = big.tile([128, 2, LQ], bf16)
            recip = []
            nrm0 = []
            att_state = {}

            # --- interleavable emission units (each ~0.5-1.1us of PE work,
            # packing several outputs per PSUM tile to relax rotation) ---
            def qu_units(h):
                st = {}

                def emit(c):
                    if c == 0:
                        st["ps"] = psp.tile([128, LQ], f32, name=f"q_ps{h}", tag="sc")
                    sl = st["ps"][:, 512 * c : 512 * c + 512]
                    for k in range(4):
                        nc.tensor.matmul(
                            sl,
                            wq_sb[:, k, 128 * h : 128 * h + 128],
                            xT[:, k, 512 * c : 512 * c + 512],
                            start=(k == 0),
                            stop=(k == 3),
                        )
                    nc.scalar.activation(
                        qT[h][:, 512 * c : 512 * c + 512],
                        sl,
                        IDN,
                        bias=bqsb[:, h : h + 1],
                    )

                return [lambda c=c: emit(c) for c in range(2)]

            def ku_units(h, g):
                # no bias: adding q.bk to every score of a q-row cancels in
                # softmax, so Wk alone is exact
                st = {}

                def emit(c):
                    if c == 0:
                        st["ps"] = psp.tile(
                            [128, LQ], f32, name=f"k_ps{h}{g}", tag="sc"
                        )
                    sl = st["ps"][:, 512 * c : 512 * c + 512]
                    for k in range(4):
                        nc.tensor.matmul(
                            sl,
                            wk_sb[:, k, 128 * h : 128 * h + 128],
                            eT[g][:, k, 512 * c : 512 * c + 512],
                            start=(k == 0),
                            stop=(k == 3),
                        )
                    nc.vector.tensor_copy(kT[h][g][:, 512 * c : 512 * c + 512], sl)

                return [lambda c=c: emit(c) for c in range(2)]

            def vu_units(g):
                st = {}

                def emit(i):
                    q = i % 4
                    if q == 0:
                        st["ps"] = psp.tile(
                            [128, LQ], f32, name=f"v_ps{g}{i}", tag="sc"
                        )
                    sl = st["ps"][:, 256 * q : 256 * q + 256]
                    for k in range(4):
                        nc.tensor.matmul(
                            sl,
                            eT[g][:, k, 128 * i : 128 * i + 128],
                            wv_sb[:, k, :],
                            start=(k == 0),
                            stop=(k == 3),
                        )
                    nc.vector.tensor_copy(v_g[g][:, i, :], sl)

                return [lambda i=i: emit(i) for i in range(8)]

            def ou_units():
                # nrm0_j = psum_h0 * r0[q] + cvec  (one fused DVE op)
                st = {}

                def emit(j):
                    if j % 2 == 0:
                        st["ps"] = psp.tile(
                            [128, LQ], f32, name=f"o_ps0_{j}", tag="sc"
                        )
                    sl = st["ps"][:, 512 * (j % 2) : 512 * (j % 2) + 512]
                    nc.tensor.matmul(
                        sl,
                        ctxT[:, 0, 128 * j : 128 * j + 128],
                        wo_sb[:, 0, :],
                        start=True,
                        stop=True,
                    )
                    n = nrm0p.tile([128, 512], f32, name=f"nrm0_{j}", tag="nrm0")
                    nc.vector.scalar_tensor_tensor(
                        n[:], sl, recip[0][:, j : j + 1], cvsb[:], MUL, ADD
                    )
                    nrm0.append(n)

                return [lambda j=j: emit(j) for j in range(8)]

            def emit_mm2(st, stop):
                lv, et_p, ktp = st.pop("pend")
                for c in range(2):
                    nc.tensor.matmul(
                        st["ps_ctx"][:, 512 * c : 512 * c + 512],
                        lv,
                        et_p[:, 512 * c : 512 * c + 512],
                        start=(ktp == 0),
                        stop=stop,
                    )

            def attn_segment(h, g, jit=None, extra=()):
                if g == 0:
                    att_state[h] = {
                        "ps_ctx": ps_c.tile(
                            [128, LQ], f32, name=f"ctx{h}", tag="ctx"
                        ),
                        "levels": [None] * 6,
                    }
                st = att_state[h]
                levels = st["levels"]
                extra = list(extra)
                for ti, kt in enumerate(range(8 * g, 8 * g + 8)):
                    ps_sc = psp.tile([128, LQ], f32, name=f"sc{h}_{kt}", tag="sc")
                    lk = kT[h][kt // 8][:, 128 * (kt % 8) : 128 * (kt % 8) + 128]
                    for c in range(2):
                        nc.tensor.matmul(
                            ps_sc[:, 512 * c : 512 * c + 512],
                            lk,
                            qT[h][:, 512 * c : 512 * c + 512],
                            start=True,
                            stop=True,
                        )
                    et_t = expp.tile([128, LQ], bf16, name=f"et{h}_{kt}", tag="et")
                    nc.scalar.activation(et_t[:], ps_sc[:], EXP, scale=SCALE)
                    lv = v_g[kt // 8][:, kt % 8, 128 * h : 128 * h + 128]
                    # defer-by-1: emit MM2 of the PREVIOUS tile after this
                    # tile's MM1s, so the PE never waits on exp(t)
                    if "pend" in st:
                        emit_mm2(st, False)
                    st["pend"] = (lv, et_t, kt)
                    if jit is not None:
                        jit[ti]()
                    # spread the remaining fill units over the segment
                    n_emit = -(-len(extra) // (8 - ti)) if extra else 0
                    for _ in range(n_emit):
                        extra.pop(0)()
                    # denominator tree (bf16 pairwise binary counter); at kt30
                    # force-collapse so only one add remains after the last exp
                    if kt == 31:
                        st["last_et"] = et_t
                    else:
                        cur, lvl = et_t, 0
                        while levels[lvl] is not None:
                            nxt = treep.tile(
                                [128, LQ], bf16, name=f"tr{h}_{kt}_{lvl}", tag="tr"
                            )
                            nc.vector.tensor_add(nxt[:], levels[lvl][:], cur[:])
                            levels[lvl] = None
                            cur, lvl = nxt, lvl + 1
                        levels[lvl] = cur
                        if kt == 30:
                            cur = levels[0]
                            for lvl in range(1, 5):
                                nxt = treep.tile(
                                    [128, LQ], bf16, name=f"tc{h}_{lvl}", tag="tr"
                                )
                                nc.vector.tensor_add(nxt[:], levels[lvl][:], cur[:])
                                levels[lvl] = None
                                cur = nxt
                            levels[0] = None
                            st["S30"] = cur
                assert not extra

            def attn_finish_a(h):
                st = att_state[h]
                emit_mm2(st, True)  # flush kt31's MM2 with stop
                if h == 1:
                    # ACT copy: frees the DVE for the denominator chain
                    nc.scalar.copy(ctxT[:, h, :], st["ps_ctx"][:])
                else:
                    nc.vector.tensor_copy(ctxT[:, h, :], st["ps_ctx"][:])
                acc = treep.tile([128, LQ], bf16, name=f"accf{h}", tag="tr")
                nc.vector.tensor_add(acc[:], st["S30"][:], st["last_et"][:])
                st["acc"] = acc

            def attn_finish_b(h):
                st = att_state[h]
                acc = st["acc"]
                den = smal.tile([128, 8], f32, name=f"den{h}", tag="den")
                pt = psp.tile([128, LQ], bf16, name=f"dt{h}", tag="sc")
                for jj in range(8):
                    nc.tensor.transpose(
                        pt[:, 128 * jj : 128 * jj + 128],
                        acc[:, 128 * jj : 128 * jj + 128],
                        identb[:],
                    )
                nc.vector.tensor_reduce(
                    den[:, 0:8],
                    pt[:].rearrange("p (j q) -> p j q", j=8),
                    axis=mybir.AxisListType.X,
                    op=mybir.AluOpType.add,
                )
                rc = smal.tile([128, 8], f32, name=f"rc{h}", tag="rc")
                nc.vector.reciprocal(rc[:], den[:])
                recip.append(rc)

            # --- emission schedule: minimal upfront block, everything else
            # interleaved into the attention tile stream ---
            for u in qu_units(0):
                u()
            ku00 = ku_units(0, 0)
            ku00[0]()

            vu0 = vu_units(0)
            ku01 = ku_units(0, 1)
            vu1 = vu_units(1)
            # e2 rides the ACT queue after the q-bias copies (~9us)
            ld_e(2, 0, eng=nc.scalar)
            ld_e(2, 1, eng=nc.scalar)
            nc.scalar.dma_start(
                wo_sb[:], wot.ap().rearrange("p (k d) -> p k d", k=2)
            )
            cvst = const.tile([128, D], f32)
            nc.scalar.dma_start(cvst[0:1, :], cvec.ap().unsqueeze(0))
            attn_segment(
                0,
                0,
                jit=vu0,
                extra=[ku00[1]] + vu1[0:2] + ku01 + vu1[2:8],
            )
            ld_e(3, 0, eng=nc.scalar)
            ld_e(3, 1, eng=nc.scalar)
            attn_segment(
                0,
                1,
                extra=ku_units(1, 0) + ku_units(0, 2) + vu_units(2),
            )
            attn_segment(
                0,
                2,
                extra=ku_units(1, 1) + qu_units(1) + ku_units(0, 3) + vu_units(3),
            )
            attn_segment(0, 3, extra=ku_units(1, 2))

            cvsb = const.tile([128, D], f32)
            nc.gpsimd.partition_broadcast(cvsb[:], cvst[0:1, :])

            attn_finish_a(0)
            ou = ou_units()
            attn_segment(1, 0, extra=ku_units(1, 3))
            attn_segment(1, 1, extra=[lambda: attn_finish_b(0)] + ou[0:4])
            attn_segment(1, 2, extra=ou[4:8])
            attn_segment(1, 3)
            attn_finish_a(1)
            attn_finish_b(1)

            # head-1 out-projection, fused combine, store (natural q order)
            stt = {}
            for j in range(8):
                if j % 2 == 0:
                    stt["ps"] = psp.tile([128, LQ], f32, name=f"o_ps1_{j}", tag="sc")
                sl = stt["ps"][:, 512 * (j % 2) : 512 * (j % 2) + 512]
                nc.tensor.matmul(
                    sl,
                    ctxT[:, 1, 128 * j : 128 * j + 128],
                    wo_sb[:, 1, :],
                    start=True,
                    stop=True,
                )
                ob = osb.tile([128, 512], f32, name=f"ob{j}", tag="ob")
                nc.vector.scalar_tensor_tensor(
                    ob[:], sl, recip[1][:, j : j + 1], nrm0[j][:], MUL, ADD
                )
                nc.sync.dma_start(outp.ap()[128 * j : 128 * j + 128, :], ob[:])

    nc.compile()
    return nc


def _get_nc():
    if "nc" not in _compiled:
        _compiled["nc"] = _build()
    return _compiled["nc"]


def _warr(wt, k):
    """[k*128, n] -> [128, k*n] bf16 so partition p reads one contiguous block."""
    import ml_dtypes

    n = wt.shape[1]
    return np.ascontiguousarray(
        wt.reshape(k, 128, n).transpose(1, 0, 2).reshape(128, k * n)
    ).astype(ml_dtypes.bfloat16)


def _make_in_maps(x, encoder_feats, Wq, Wk, Wv, bq, bk, bv, Wo, bo):
    import ml_dtypes

    f = np.float32
    bf = ml_dtypes.bfloat16
    x = np.asarray(x, f)
    encoder_feats = np.asarray(encoder_feats, f)
    Wq, Wk, Wv, Wo = (np.asarray(a, f) for a in (Wq, Wk, Wv, Wo))
    bq, bk, bv, bo = (np.asarray(a, f) for a in (bq, bk, bv, bo))
    xts = [np.ascontiguousarray(x[b].T).astype(bf) for b in range(B)]
    ets = [np.ascontiguousarray(encoder_feats[b].T).astype(bf) for b in range(B)]
    in_maps = []
    for c in range(NCORES):
        b, hp = c // 2, c % 2
        sl = slice(256 * hp, 256 * hp + 256)
        cv = Wo[:, sl] @ bv[sl]
        if hp == 0:
            cv = cv + bo
        in_maps.append(
            {
                "xt": xts[b],
                "et": ets[b],
                "wqt": _warr(Wq[sl, :].T, 4),
                "wkt": _warr(Wk[sl, :].T, 4),
                "wvt": _warr(Wv[sl, :].T, 4),
                "wot": _warr(Wo[:, sl].T, 2),
                "bq2": np.ascontiguousarray(bq[sl].reshape(2, 128).T),
                "cvec": np.ascontiguousarray(cv, dtype=f),
            }
        )
    return in_maps


def kernel(x, encoder_feats, Wq, Wk, Wv, bq, bk, bv, Wo, bo, _trace=False):
    from concourse.bass_utils import run_bass_kernel_spmd

    nc = _get_nc()
    in_maps = _make_in_maps(x, encoder_feats, Wq, Wk, Wv, bq, bk, bv, Wo, bo)
    kw = {}
    if _trace:
        kw = dict(trace=True, trace_cores=[0])
    res = run_bass_kernel_spmd(nc, in_maps, core_ids=list(range(NCORES)), **kw)
    _compiled["last_res"] = res
    out = np.empty((B, LQ, D), np.float32)
    for b in range(B):
        out[b] = res.results[2 * b]["outp"] + res.results[2 * b + 1]["outp"]
    return out


# revision 9
# speedup vs baseline: 1.1636x; 1.1636x over previous
"""Trainium2 Bass kernel for nn_AttentionFusion (cross-attention, B=4, LQ=1024,
LKV=4096, D=512, H=4 heads of 128).

Sharding: 8 cores = (batch b in 0..3) x (head-pair hp in 0..1). Core c = 2*b+hp
computes attention for heads {2hp, 2hp+1} of batch b plus its partial
out-projection (tensor-parallel split of Wo). Host sums the two partials per
batch (the TP un-shard); everything else runs on device in bf16 with fp32
accumulation.

v4: activations pre-transposed + pre-cast to bf16 on host (no on-chip
transposes, contiguous loads, halved DMA). bk dropped (a per-q additive score
shift cancels in softmax). All projection / out-projection work is emitted as
small "units" interleaved into the attention tile stream so the PE never
idles on the exp engine (exp ~1.0us/tile vs 864ns of attention MMs). Non-head
loads ride the ACT queue as staggered triggers so the first x/enc chunks get
full DMA bandwidth. Denominator tree pre-collapses at kt30, transposes run in
bf16, and the tail fuses (psum*recip)+carry in one DVE op per q-tile.

Per-core dataflow:
  xT [e,q], eT [e,kv]   <- direct chunked DMA (already bf16+transposed)
  qT [d,q]              <- weight-stationary projection; bq fused on ACT
  kT [d,kv]             <- weight-stationary projection (no bias; DVE copy)
  v  [kv,d]             <- encoder-stationary projection (bv folded into cvec)
  scoresT [kv,q] (PSUM) = kT-tile.T @ qT ; P = exp(scale*scoresT) on ACT (bf16)
  ctx~T [d,q]  (PSUM)  += v-tile.T @ P  over kv tiles (unnormalized, 1-deep
                          software pipeline: MM2(t) emitted after MM1(t+1))
  denom: bf16 pairwise tree of P tiles on DVE -> PE-transpose (bf16) ->
         free-dim reduce -> reciprocal (per-partition [q,1] layout)
  out[q,e] = (ctx~T.T @ Wo_h.T) * recip_h[q] (+ cvec)  summed over h, f32
"""

import numpy as np

B, LQ, LKV, D, H, HD = 4, 1024, 4096, 512, 4, 128
NCORES = 8
SCALE = 1.0 / float(np.sqrt(HD))

_compiled = {}


def _build():
    import concourse.bacc as bacc
    import concourse.mybir as mybir
    from concourse import tile
    from concourse.masks import make_identity

    bf16, f32 = mybir.dt.bfloat16, mybir.dt.float32
    EXP = mybir.ActivationFunctionType.Exp
    IDN = mybir.ActivationFunctionType.Identity
    MUL = mybir.AluOpType.mult
    ADD = mybir.AluOpType.add

    nc = bacc.Bacc(
        "TRN2",
        target_bir_lowering=False,
        debug=False,
        enable_asserts=True,
        num_devices=NCORES,
    )

    xt = nc.dram_tensor("xt", [D, LQ], bf16, kind="ExternalInput")
    et = nc.dram_tensor("et", [D, LKV], bf16, kind="ExternalInput")
    wqt = nc.dram_tensor("wqt", [128, 1024], bf16, kind="ExternalInput")
    wkt = nc.dram_tensor("wkt", [128, 1024], bf16, kind="ExternalInput")
    wvt = nc.dram_tensor("wvt", [128, 1024], bf16, kind="ExternalInput")
    wot = nc.dram_tensor("wot", [128, 1024], bf16, kind="ExternalInput")
    bq2 = nc.dram_tensor("bq2", [128, 2], f32, kind="ExternalInput")
    cvec = nc.dram_tensor("cvec", [D], f32, kind="ExternalInput")
    outp = nc.dram_tensor("outp", [LQ, D], f32, kind="ExternalOutput")

    with tile.TileContext(nc) as tc:
        with (
            tc.tile_pool(name="const", bufs=1) as const,
            tc.tile_pool(name="big", bufs=1) as big,
            tc.tile_pool(name="expp", bufs=6) as expp,
            tc.tile_pool(name="tree", bufs=7) as treep,
            tc.tile_pool(name="smal", bufs=4) as smal,
            tc.tile_pool(name="nrm0p", bufs=8) as nrm0p,
            tc.tile_pool(name="osb", bufs=4) as osb,
            tc.tile_pool(name="ps", bufs=3, space="PSUM") as psp,
            tc.tile_pool(name="ps_c", bufs=1, space="PSUM") as ps_c,
        ):
            wq_sb = const.tile([128, 4, 256], bf16)
            wk_sb = const.tile([128, 4, 256], bf16)
            wv_sb = const.tile([128, 4, 256], bf16)
            wo_sb = const.tile([128, 2, D], bf16)
            bqsb = const.tile([128, 2], f32)

            xT = big.tile([128, 4, LQ], bf16)
            eT = [
                big.tile([128, 4, 1024], bf16, tag=f"eT{g}", name=f"eT{g}")
                for g in range(4)
            ]

            def ld_x(c):
                nc.sync.dma_start(
                    xT[:, :, 512 * c : 512 * c + 512],
                    xt.ap()[:, 512 * c : 512 * c + 512].rearrange(
                        "(k p) q -> p k q", k=4
                    ),
                )

            def ld_e(g, c, eng=None):
                lo = 1024 * g + 512 * c
                (eng or nc.sync).dma_start(
                    eT[g][:, :, 512 * c : 512 * c + 512],
                    et.ap()[:, lo : lo + 512].rearrange("(k p) kv -> p k kv", k=4),
                )

            # sync queue: only what the head critically needs (plus e1c0)
            nc.sync.dma_start(wq_sb[:], wqt.ap().rearrange("p (k d) -> p k d", k=4))
            ld_x(0)
            nc.sync.dma_start(wk_sb[:], wkt.ap().rearrange("p (k d) -> p k d", k=4))
            ld_e(0, 0)
            nc.sync.dma_start(bqsb[:], bq2[:])
            ld_x(1)
            ld_e(0, 1)
            nc.sync.dma_start(wv_sb[:], wvt.ap().rearrange("p (k d) -> p k d", k=4))
            ld_e(1, 0)

            # --- constants ---
            ones = const.tile([128, 1], f32)
            nc.vector.memset(ones[:], 1.0)
            identb = const.tile([128, 128], bf16)
            make_identity(nc, identb[:])
            # warm the ACT exp table set early (~2.7us table load)
            warm = const.tile([128, 1], f32)
            nc.scalar.activation(warm[:], ones[:], EXP)
            # dummy matmul burst during the DMA wait: ~4us of PE activity
            # flips the HAM clock gate to 8/8 before the first real matmuls
            wps = psp.tile([128, LQ], f32, name="warm_ps", tag="sc")
            for _ in range(40):
                nc.tensor.matmul(
                    wps[:, 0:128], identb[:], identb[:], start=True, stop=True
                )
            # remaining loads ride the ACT queue: each trigger fires when the
            # exp stream reaches it, keeping early DMA bandwidth for x/e0
            ld_e(1, 1, eng=nc.scalar)

            qT = [
                big.tile([128, LQ], bf16, tag=f"qT{h}", name=f"qT{h}")
                for h in range(2)
            ]
            kT = [
                [
                    big.tile([128, 1024], bf16, tag=f"kT{h}_{g}", name=f"kT{h}_{g}")
                    for g in range(4)
                ]
                for h in range(2)
            ]
            v_g = [
                big.tile([128, 8, 256], bf16, tag=f"v{g}", name=f"v{g}")
                for g in range(4)
            ]
            ctxT = big.tile([128, 2, LQ], bf16)
            recip = []
            nrm0 = []
            att_state = {}

            # --- interleavable emission units (each ~0.5-1.1us of PE work,
            # packing several outputs per PSUM tile to relax rotation) ---
            def qu_units(h):
                st = {}

                def emit(c):
                    if c == 0:
                        st["ps"] = psp.tile([128, LQ], f32, name=f"q_ps{h}", tag="sc")
                    sl = st["ps"][:, 512 * c : 512 * c + 512]
                    for k in range(4):
                        nc.tensor.matmul(
                            sl,
                            wq_sb[:, k, 128 * h : 128 * h + 128],
                            xT[:, k, 512 * c : 512 * c + 512],
                            start=(k == 0),
                            stop=(k == 3),
                        )
                    nc.scalar.activation(
                        qT[h][:, 512 * c : 512 * c + 512],
                        sl,
                        IDN,
                        bias=bqsb[:, h : h + 1],
                    )

                return [lambda c=c: emit(c) for c in range(2)]

            def ku_units(h, g):
                # no bias: adding q.bk to every score of a q-row cancels in
                # softmax, so Wk alone is exact
                st = {}

                def emit(c):
                    if c == 0:
                        st["ps"] = psp.tile(
                            [128, LQ], f32, name=f"k_ps{h}{g}", tag="sc"
                        )
                    sl = st["ps"][:, 512 * c : 512 * c + 512]
                    for k in range(4):
                        nc.tensor.matmul(
                            sl,
                            wk_sb[:, k, 128 * h : 128 * h + 128],
                            eT[g][:, k, 512 * c : 512 * c + 512],
                            start=(k == 0),
                            stop=(k == 3),
                        )
                    nc.vector.tensor_copy(kT[h][g][:, 512 * c : 512 * c + 512], sl)

                return [lambda c=c: emit(c) for c in range(2)]

            def vu_units(g):
                st = {}

                def emit(i):
                    q = i % 4
                    if q == 0:
                        st["ps"] = psp.tile(
                            [128, LQ], f32, name=f"v_ps{g}{i}", tag="sc"
                        )
                    sl = st["ps"][:, 256 * q : 256 * q + 256]
                    for k in range(4):
                        nc.tensor.matmul(
                            sl,
                            eT[g][:, k, 128 * i : 128 * i + 128],
                            wv_sb[:, k, :],
                            start=(k == 0),
                            stop=(k == 3),
                        )
                    nc.vector.tensor_copy(v_g[g][:, i, :], sl)

                return [lambda i=i: emit(i) for i in range(8)]

            def ou_units():
                # nrm0_j = psum_h0 * r0[q] + cvec  (one fused DVE op)
                st = {}

                def emit(j):
                    if j % 2 == 0:
                        st["ps"] = psp.tile(
                            [128, LQ], f32, name=f"o_ps0_{j}", tag="sc"
                        )
                    sl = st["ps"][:, 512 * (j % 2) : 512 * (j % 2) + 512]
                    nc.tensor.matmul(
                        sl,
                        ctxT[:, 0, 128 * j : 128 * j + 128],
                        wo_sb[:, 0, :],
                        start=True,
                        stop=True,
                    )
                    n = nrm0p.tile([128, 512], f32, name=f"nrm0_{j}", tag="nrm0")
                    nc.vector.scalar_tensor_tensor(
                        n[:], sl, recip[0][:, j : j + 1], cvsb[:], MUL, ADD
                    )
                    nrm0.append(n)

                return [lambda j=j: emit(j) for j in range(8)]

            def emit_mm2(st, stop):
                lv, et_p, ktp = st.pop("pend")
                for c in range(2):
                    nc.tensor.matmul(
                        st["ps_ctx"][:, 512 * c : 512 * c + 512],
                        lv,
                        et_p[:, 512 * c : 512 * c + 512],
                        start=(ktp == 0),
                        stop=stop,
                    )

            def attn_segment(h, g, jit=None, extra=()):
                if g == 0:
                    att_state[h] = {
                        "ps_ctx": ps_c.tile(
                            [128, LQ], f32, name=f"ctx{h}", tag="ctx"
                        ),
                        "levels": [None] * 6,
                    }
                st = att_state[h]
                levels = st["levels"]
                extra = list(extra)
                for ti, kt in enumerate(range(8 * g, 8 * g + 8)):
                    ps_sc = psp.tile([128, LQ], f32, name=f"sc{h}_{kt}", tag="sc")
                    lk = kT[h][kt // 8][:, 128 * (kt % 8) : 128 * (kt % 8) + 128]
                    for c in range(2):
                        nc.tensor.matmul(
                            ps_sc[:, 512 * c : 512 * c + 512],
                            lk,
                            qT[h][:, 512 * c : 512 * c + 512],
                            start=True,
                            stop=True,
                        )
                    et_t = expp.tile([128, LQ], bf16, name=f"et{h}_{kt}", tag="et")
                    nc.scalar.activation(et_t[:], ps_sc[:], EXP, scale=SCALE)
                    lv = v_g[kt // 8][:, kt % 8, 128 * h : 128 * h + 128]
                    # defer-by-1: emit MM2 of the PREVIOUS tile after this
                    # tile's MM1s, so the PE never waits on exp(t)
                    if "pend" in st:
                        emit_mm2(st, False)
                    st["pend"] = (lv, et_t, kt)
                    if jit is not None:
                        jit[ti]()
                    # spread the remaining fill units over the segment
                    n_emit = -(-len(extra) // (8 - ti)) if extra else 0
                    for _ in range(n_emit):
                        extra.pop(0)()
                    # denominator tree (bf16 pairwise binary counter); at kt30
                    # force-collapse so only one add remains after the last exp
                    if kt == 31:
                        st["last_et"] = et_t
                    else:
                        cur, lvl = et_t, 0
                        while levels[lvl] is not None:
                            nxt = treep.tile(
                                [128, LQ], bf16, name=f"tr{h}_{kt}_{lvl}", tag="tr"
                            )
                            nc.vector.tensor_add(nxt[:], levels[lvl][:], cur[:])
                            levels[lvl] = None
                            cur, lvl = nxt, lvl + 1
                        levels[lvl] = cur
                        if kt == 30:
                            cur = levels[0]
                            for lvl in range(1, 5):
                                nxt = treep.tile(
                                    [128, LQ], bf16, name=f"tc{h}_{lvl}", tag="tr"
                                )
                                nc.vector.tensor_add(nxt[:], levels[lvl][:], cur[:])
                                levels[lvl] = None
                                cur = nxt
                            levels[0] = None
                            st["S30"] = cur
                assert not extra

            def attn_finish_a(h):
                st = att_state[h]
                emit_mm2(st, True)  # flush kt31's MM2 with stop
                if h == 1:
                    # ACT copy: frees the DVE for the denominator chain
                    nc.scalar.copy(ctxT[:, h, :], st["ps_ctx"][:])
                else:
                    nc.vector.tensor_copy(ctxT[:, h, :], st["ps_ctx"][:])
                acc = treep.tile([128, LQ], bf16, name=f"accf{h}", tag="tr")
                nc.vector.tensor_add(acc[:], st["S30"][:], st["last_et"][:])
                st["acc"] = acc

            def attn_finish_b(h):
                st = att_state[h]
                acc = st["acc"]
                den = smal.tile([128, 8], f32, name=f"den{h}", tag="den")
                pt = psp.tile([128, LQ], bf16, name=f"dt{h}", tag="sc")
                for jj in range(8):
                    nc.tensor.transpose(
                        pt[:, 128 * jj : 128 * jj + 128],
                        acc[:, 128 * jj : 128 * jj + 128],
                        identb[:],
                    )
                nc.vector.tensor_reduce(
                    den[:, 0:8],
                    pt[:].rearrange("p (j q) -> p j q", j=8),
                    axis=mybir.AxisListType.X,
                    op=mybir.AluOpType.add,
                )
                rc = smal.tile([128, 8], f32, name=f"rc{h}", tag="rc")
                nc.vector.reciprocal(rc[:], den[:])
                recip.append(rc)

            # --- emission schedule: minimal upfront block, everything else
            # interleaved into the attention tile stream ---
            for u in qu_units(0):
                u()
            ku00 = ku_units(0, 0)
            ku00[0]()

            vu0 = vu_units(0)
            ku01 = ku_units(0, 1)
            vu1 = vu_units(1)
            # e2 rides the ACT queue after the q-bias copies (~9us)
            ld_e(2, 0, eng=nc.scalar)
            ld_e(2, 1, eng=nc.scalar)
            nc.scalar.dma_start(
                wo_sb[:], wot.ap().rearrange("p (k d) -> p k d", k=2)
            )
            cvst = const.tile([128, D], f32)
            nc.scalar.dma_start(cvst[0:1, :], cvec.ap().unsqueeze(0))
            attn_segment(
                0,
                0,
                jit=vu0,
                extra=[ku00[1]] + vu1[0:2] + ku01 + vu1[2:8],
            )
            ld_e(3, 0, eng=nc.scalar)
            ld_e(3, 1, eng=nc.scalar)
            attn_segment(
                0,
                1,
                extra=ku_units(1, 0) + ku_units(0, 2) + vu_units(2),
            )
            attn_segment(
                0,
                2,
                extra=ku_units(1, 1) + qu_units(1) + ku_units(0, 3) + vu_units(3),
            )
            attn_segment(0, 3, extra=ku_units(1, 2))

            cvsb = const.tile([128, D], f32)
            nc.gpsimd.partition_broadcast(cvsb[:], cvst[0:1, :])

            attn_finish_a(0)
            ou = ou_units()
            attn_segment(1, 0, extra=ku_units(1, 3))
            attn_segment(1, 1, extra=[lambda: attn_finish_b(0)] + ou[0:4])
            attn_segment(1, 2, extra=ou[4:8])
            attn_segment(1, 3)
            attn_finish_a(1)
            attn_finish_b(1)

            # head-1 out-projection, fused combine, store (natural q order)
            stt = {}
            for j in range(8):
                if j % 2 == 0:
                    stt["ps"] = psp.tile([128, LQ], f32, name=f"o_ps1_{j}", tag="sc")
                sl = stt["ps"][:, 512 * (j % 2) : 512 * (j % 2) + 512]
                nc.tensor.matmul(
                    sl,
                    ctxT[:, 1, 128 * j : 128 * j + 128],
                    wo_sb[:, 1, :],
                    start=True,
                    stop=True,
                )
                ob = osb.tile([128, 512], f32, name=f"ob{j}", tag="ob")
                nc.vector.scalar_tensor_tensor(
                    ob[:], sl, recip[1][:, j : j + 1], nrm0[j][:], MUL, ADD
                )
                nc.sync.dma_start(outp.ap()[128 * j : 128 * j + 128, :], ob[:])

    nc.compile()
    return nc


def _get_nc():
    if "nc" not in _compiled:
        _compiled["nc"] = _build()
    return _compiled["nc"]


def _warr(wt, k):
    """[k*128, n] -> [128, k*n] bf16 so partition p reads one contiguous block."""
    import ml_dtypes

    n = wt.shape[1]
    return np.ascontiguousarray(
        wt.reshape(k, 128, n).transpose(1, 0, 2).reshape(128, k * n)
    ).astype(ml_dtypes.bfloat16)


def _make_in_maps(x, encoder_feats, Wq, Wk, Wv, bq, bk, bv, Wo, bo):
    import ml_dtypes

    f = np.float32
    bf = ml_dtypes.bfloat16
    x = np.asarray(x, f)
    encoder_feats = np.asarray(encoder_feats, f)
    Wq, Wk, Wv, Wo = (np.asarray(a, f) for a in (Wq, Wk, Wv, Wo))
    bq, bk, bv, bo = (np.asarray(a, f) for a in (bq, bk, bv, bo))
    xts = [np.ascontiguousarray(x[b].T).astype(bf) for b in range(B)]
    ets = [np.ascontiguousarray(encoder_feats[b].T).astype(bf) for b in range(B)]
    in_maps = []
    for c in range(NCORES):
        b, hp = c // 2, c % 2
        sl = slice(256 * hp, 256 * hp + 256)
        cv = Wo[:, sl] @ bv[sl]
        if hp == 0:
            cv = cv + bo
        in_maps.append(
            {
                "xt": xts[b],
                "et": ets[b],
                "wqt": _warr(Wq[sl, :].T, 4),
                "wkt": _warr(Wk[sl, :].T, 4),
                "wvt": _warr(Wv[sl, :].T, 4),
                "wot": _warr(Wo[:, sl].T, 2),
                "bq2": np.ascontiguousarray(bq[sl].reshape(2, 128).T),
                "cvec": np.ascontiguousarray(cv, dtype=f),
            }
        )
    return in_maps


def kernel(x, encoder_feats, Wq, Wk, Wv, bq, bk, bv, Wo, bo, _trace=False):
    from concourse.bass_utils import run_bass_kernel_spmd

    nc = _get_nc()
    in_maps = _make_in_maps(x, encoder_feats, Wq, Wk, Wv, bq, bk, bv, Wo, bo)
    kw = {}
    if _trace:
        kw = dict(trace=True, trace_cores=[0])
    res = run_bass_kernel_spmd(nc, in_maps, core_ids=list(range(NCORES)), **kw)
    _compiled["last_res"] = res
    out = np.empty((B, LQ, D), np.float32)
    for b in range(B):
        out[b] = res.results[2 * b]["outp"] + res.results[2 * b + 1]["outp"]
    return out
